# revision 37
# baseline (speedup 1.0000x reference)
"""Trainium2 Bass kernel for nn_DynamicConv2d: per-sample dynamic conv.

  feat = x.mean(H,W); h1 = relu(feat@w1+b1); wgen = (h1@w2+b2) -> per-sample
  [COUT, CIN, 3, 3] conv weights; out[s] = conv2d(x[s], wgen[s], pad=1).

Sharding: batch B=32 across 8 cores (4 samples/core), MLP params replicated.

Per-core design (v2):
  - conv uses a 6-pass h-parity scheme: out partitions M=128=(hp,co) for ONE
    sample, contraction K=128=(j,ci) where j indexes two consecutive image
    rows.  x is stored per-sample as [128=(j,ci), 130, 130] bf16 with the
    j=1 half a one-row-shifted duplicate (DVE cross-partition copy), so one
    matmul contracts 2 rows x 64 ci.  Per (dx, rho) pass 3 of 4 (j,hp)
    quadrants are useful -> 6 passes/tile vs 9 for the sample-paired block
    diagonal scheme: 196608 PE cycles/core instead of 294912.
  - w2 ships as fp8 e4m3 (halves its DMA), column-reordered tap-major so the
    weight-generator emits per-tap chunks.  wgen matmuls use the w2 chunk as
    the STATIONARY [128 hid, 128=(colow,ci)] and h1 as the moving tensor, so
    psum lands already ci-on-partitions - no transposes.  h1 is split into
    hi+lo e4m3 parts (2 accumulating matmuls) to keep quantization error low
    (measured end-to-end rel err ~1.2e-2 < 2e-2).
  - conv stationaries [K=(j,ci), (s,hp,co)] are assembled from tap psums +
    host-prepared b2 stationaries by 36 strided DVE adds per pass.
  - schedule: x(s0,s1) j0-halves -> w2 (wgen pass A for s0,s1 trails the
    chunks) -> x(s2,s3); conv(s0) starts right after w2; pass B for s2,s3 is
    interleaved into the conv instruction stream; drains on ScalarE; output
    written bf16 in [s, hp, co, r, w] layout (1KB descriptors), host
    reassembles.
"""

import sys

for _p in ("/opt/trn_rl_repo",):
    if _p not in sys.path:
        sys.path.insert(0, _p)

from contextlib import ExitStack

import numpy as np

import concourse.bass as bass
import concourse.tile as tile
from concourse import bacc, mybir
from concourse.bass_utils import run_bass_kernel_spmd
from concourse.tile_rust import add_dep_helper

F32 = mybir.dt.float32
BF16 = mybir.dt.bfloat16
FP8 = mybir.dt.float8e4

B, CIN, COUT, K, H, W = 32, 64, 64, 3, 128, 128
NCORES = 8
BSH = B // NCORES          # 4 samples per core
HID = 128                  # MLP hidden
JTOT = COUT * CIN * K * K  # 36864
NTAP = K * K               # 9
HW = H * W
WP = W + 2                 # width-padded image
MP = H + 2                 # row-padded storage (m = h + 1 on j0 half)
R2 = H // 2                # 64 row-pairs

NTAPCH = JTOT // NTAP      # 4096 w2 columns per tap
NCHUNK = 32                # (cohi) chunks per tap, 128 cols each

# tap (dy) -> [(rho, j, hp), ...] stationary slots it fills
DY_SLOTS = {
    0: [(0, 0, 0), (0, 1, 1)],
    1: [(0, 1, 0), (1, 0, 1)],
    2: [(1, 0, 0), (1, 1, 1)],
}

TROWS = 4                  # row-pairs per conv tile -> N = 4*128 = 512
NT = R2 // TROWS           # 16 conv tiles per sample
TB = 2                     # conv tiles per output DMA


def build_kernel_body(nc, tc, ctx, aps):
    x_ap = aps["x"]        # [BSH, CIN, H, WP] bf16 (width-padded)
    w1_ap = aps["w1"]      # [CIN, HID] f32 (pre-scaled by 1/HW)
    b1_ap = aps["b1"]      # [HID, 1] f32
    w2_ap = aps["w2"]      # [HID, JTOT] e4m3, tap-major reordered
    b2s_ap = aps["b2s"]    # [128, 6, 512] bf16 stationary-layout bias
    out_ap = aps["out"]    # [BSH, 2, COUT, R2, W] bf16

    const = ctx.enter_context(tc.tile_pool(name="const", bufs=1))
    xpool = ctx.enter_context(tc.tile_pool(name="xpool", bufs=1))
    w2pool = ctx.enter_context(tc.tile_pool(name="w2pool", bufs=1))
    fpool = ctx.enter_context(tc.tile_pool(name="fpool", bufs=2))
    outp = ctx.enter_context(tc.tile_pool(name="outp", bufs=2))
    mlp_ps = ctx.enter_context(tc.tile_pool(name="mlp_ps", bufs=1, space="PSUM"))
    wg_ps = ctx.enter_context(tc.tile_pool(name="wg_ps", bufs=2, space="PSUM"))
    cv_ps = ctx.enter_context(tc.tile_pool(name="cv_ps", bufs=5, space="PSUM"))

    # ---- small constants ----
    w1_sb = const.tile([CIN, HID], F32)
    nc.sync.dma_start(out=w1_sb, in_=w1_ap)
    b1_sb = const.tile([HID, 1], F32)
    nc.sync.dma_start(out=b1_sb, in_=b1_ap)
    b2s = const.tile([128, 6, 512], BF16)
    nc.sync.dma_start(out=b2s, in_=b2s_ap)

    # conv stationaries [K=(j,ci), 6 passes, (s, hp, co)]
    stat = const.tile([128, 6, 512], BF16)
    # zero slots: rho0 -> (j0, hp1); rho1 -> (j1, hp0)
    for dx in range(3):
        nc.vector.memset(stat[0:64, dx * 2 + 0, :].rearrange(
            "p (s hp co) -> p s hp co", s=BSH, hp=2, co=COUT)[:, :, 1, :], 0.0)
        nc.vector.memset(stat[64:128, dx * 2 + 1, :].rearrange(
            "p (s hp co) -> p s hp co", s=BSH, hp=2, co=COUT)[:, :, 0, :], 0.0)

    # per-sample x tiles [(j,ci), m, WP]; j0: m = h+1 (rows -1..128),
    # j1: m = h (rows 0..127, m=128 zero)
    xd = [xpool.tile([128, MP, WP], BF16, name=f"xd{s}") for s in range(BSH)]

    # w2 tap chunks stay resident (pass B re-reads them)
    w2c = [w2pool.tile([HID, NTAPCH], FP8, name=f"w2c{t}") for t in range(NTAP)]

    # feat partials: [64, BSH*4] chunk sums -> feat4 [64, BSH]
    fpart = const.tile([CIN, BSH * 4], F32)
    fparth = const.tile([CIN, BSH * 4], F32)
    feat4 = const.tile([CIN, BSH], F32)
    # h1 hi/lo e4m3 [HID, 2, BSH]
    h1f = const.tile([HID, BSH], F32)
    h1hf = const.tile([HID, BSH], F32)
    h1T8 = const.tile([HID, 2, BSH], FP8)

    NXC = 4              # x DMA/feat chunks per sample
    XROWS = H // NXC     # 32 rows per chunk

    def load_x_j0(s):
        for c in range(NXC):
            r0 = c * XROWS
            nc.sync.dma_start(
                out=xd[s][0:64, 1 + r0 : 1 + r0 + XROWS, :],
                in_=x_ap[s, :, r0 : r0 + XROWS, :],
            )

    def load_x_j1_dma(s, chunks=None):
        for c in (range(NXC) if chunks is None else chunks):
            r0 = c * XROWS
            nc.sync.dma_start(
                out=xd[s][64:128, r0 : r0 + XROWS, :],
                in_=x_ap[s, :, r0 : r0 + XROWS, :],
            )

    def dup_j1(s, eng, chunks=None):
        # row-shifted cross-partition copy j0 -> j1
        e = nc.vector if eng == "vector" else nc.gpsimd
        for c in (range(NXC) if chunks is None else chunks):
            r0 = c * XROWS
            e.tensor_copy(
                out=xd[s][64:128, r0 : r0 + XROWS, :],
                in_=xd[s][0:64, 1 + r0 : 1 + r0 + XROWS, :],
            )

    def edge_memset(s):
        nc.vector.memset(xd[s][0:64, 0:1, :], 0.0)    # j0 row h=-1
        nc.vector.memset(xd[s][64:128, 128:130, :], 0.0)  # j1 rows h=128,129

    # feat: per-chunk sums; scalar = activation accum; vector = bf16 tree
    # adds (DVE 2-byte perf modes make adds ~3x cheaper than tensor_reduce)
    def feat_chunk(s, c, eng):
        r0 = c * XROWS
        src = xd[s][0:64, 1 + r0 : 1 + r0 + XROWS, :]
        dst = fpart[:, s * NXC + c : s * NXC + c + 1]
        if eng == "scalar":
            ascr = fpool.tile([CIN, XROWS * WP], BF16, tag="ascr", bufs=1,
                              name=f"ascr{s}_{c}")
            nc.scalar.activation(
                out=ascr, in_=src,
                func=mybir.ActivationFunctionType.Copy,
                accum_out=dst,
            )
        else:
            feat_tree_level(s, c, 0)
            for lv in range(1, 6):
                feat_tree_level(s, c, lv)

    # tree level lv for chunk (s, c); lv 5 = final reduce into fpart
    ftree_tiles = {}

    def feat_tree_level(s, c, lv):
        r0 = c * XROWS
        src = xd[s][0:64, 1 + r0 : 1 + r0 + XROWS, :]
        key = (s, c)
        if lv == 0:
            tr = fpool.tile([CIN, XROWS // 2, WP], BF16, tag="ftree", bufs=3,
                            name=f"ftree{s}_{c}")
            ftree_tiles[key] = tr
            nc.vector.tensor_add(
                tr, src[:, 0 : XROWS // 2, :],
                src[:, XROWS // 2 : XROWS, :],
            )
        elif lv < 5:
            tr = ftree_tiles[key]
            w = XROWS // 2 >> (lv - 1)
            h = w // 2
            nc.vector.tensor_add(
                tr[:, 0:h, :], tr[:, 0:h, :], tr[:, h:w, :]
            )
        else:
            tr = ftree_tiles[key]
            nc.vector.tensor_reduce(
                out=fpart[:, s * NXC + c : s * NXC + c + 1],
                in_=tr[:, 0:1, :], axis=mybir.AxisListType.XY,
                op=mybir.AluOpType.add,
            )

    def feat_trees_interleaved(s):
        # emit level-by-level across the 4 chunks so independent chains
        # pipeline on DVE
        for lv in range(6):
            for c in range(NXC):
                feat_tree_level(s, c, lv)

    def feat_combine(s):
        nc.vector.tensor_reduce(
            out=feat4[:, s : s + 1],
            in_=fpart[:, s * NXC : (s + 1) * NXC],
            axis=mybir.AxisListType.X,
            op=mybir.AluOpType.add,
        )

    def h1_compute(s_lo, s_hi, relu_eng="scalar", dep=None):
        # h1 = relu(w1s.T @ feat + b1); split into hi+lo e4m3
        n = s_hi - s_lo
        ps = mlp_ps.tile([HID, BSH], F32, tag="h1ps", name=f"h1ps{s_lo}")
        mm = nc.tensor.matmul(
            out=ps[:, 0:n], lhsT=w1_sb, rhs=feat4[:, s_lo:s_hi],
            start=True, stop=True,
        )
        if dep is not None:
            add_dep_helper(mm.ins, dep.ins, sync=True,
                           reason="h1B after conv-s1")
        if relu_eng == "scalar":
            nc.scalar.activation(
                out=h1f[:, s_lo:s_hi], in_=ps[:, 0:n],
                func=mybir.ActivationFunctionType.Relu,
                bias=b1_sb, scale=1.0,
            )
        else:
            # keep ScalarE free: bias-add + relu on DVE
            nc.vector.tensor_scalar(
                out=h1f[:, s_lo:s_hi], in0=ps[:, 0:n],
                scalar1=b1_sb, scalar2=0.0,
                op0=mybir.AluOpType.add, op1=mybir.AluOpType.max,
            )
        nc.vector.tensor_copy(out=h1T8[:, 0, s_lo:s_hi], in_=h1f[:, s_lo:s_hi])
        nc.vector.tensor_copy(out=h1hf[:, s_lo:s_hi], in_=h1T8[:, 0, s_lo:s_hi])
        nc.vector.tensor_sub(
            h1f[:, s_lo:s_hi], h1f[:, s_lo:s_hi], h1hf[:, s_lo:s_hi]
        )
        nc.vector.tensor_copy(out=h1T8[:, 1, s_lo:s_hi], in_=h1f[:, s_lo:s_hi])

    # ---- wgen pass for samples [s_lo, s_hi): tap matmuls + assembly ----
    # two accumulating matmuls per chunk (h1 hi then lo into the same psum
    # region - PE is idle here, DVE is the scarce engine), then
    # stat[(j,ci), dxr, (s,hp,co)] = psum + b2 via strided DVE adds.
    def wgen_tap(tap, s_lo, s_hi, pass_id, dep=None):
        tp = wg_ps.tile([128, NCHUNK * BSH], F32, tag="tap",
                        name=f"tap{pass_id}_{tap}")
        for ch in range(NCHUNK):
            o = ch * BSH
            lhsT = w2c[tap][:, ch * 128 : (ch + 1) * 128]
            m1 = nc.tensor.matmul(
                out=tp[:, o : o + BSH], lhsT=lhsT, rhs=h1T8[:, 0, :],
                start=True, stop=False,
            )
            m2 = nc.tensor.matmul(
                out=tp[:, o : o + BSH], lhsT=lhsT, rhs=h1T8[:, 1, :],
                start=False, stop=True,
            )
            if dep is not None:
                add_dep_helper(m1.ins, dep.ins, sync=True,
                               reason="pass-B after conv-s1")
                add_dep_helper(m2.ins, dep.ins, sync=True,
                               reason="pass-B after conv-s1")
        dy, dx = tap // 3, tap % 3
        statv = stat.rearrange("p d (s hp co) -> p d s hp co",
                               s=BSH, hp=2, co=COUT)
        b2v = b2s.rearrange("p d (s hp co) -> p d s hp co",
                            s=BSH, hp=2, co=COUT)
        tpv = tp.rearrange("p (c s) -> p s c", c=NCHUNK, s=BSH)
        for rho, j, hp in DY_SLOTS[dy]:
            dxr = dx * 2 + rho
            for colow in range(2):
                dst = statv[j * 64 : (j + 1) * 64, dxr, s_lo:s_hi, hp,
                            colow : COUT : 2]
                nc.vector.tensor_add(
                    dst,
                    tpv[colow * 64 : (colow + 1) * 64, s_lo:s_hi, :],
                    b2v[j * 64 : (j + 1) * 64, dxr, s_lo:s_hi, hp,
                        colow : COUT : 2],
                )

    def load_w2(tap):
        nc.sync.dma_start(
            out=w2c[tap], in_=w2_ap[:, tap * NTAPCH : (tap + 1) * NTAPCH]
        )

    # ---- conv for one sample, tiles [t_lo, t_hi); optional callback after
    # each tile-pair to interleave other PE work into the program order ----
    last_mm = [None]

    def conv_tiles(s, t_lo, t_hi, after_pair=None):
        for tb in range(t_lo // TB, t_hi // TB):
            ost = outp.tile([128, TB * TROWS * W], BF16, tag="ost",
                            name=f"ost{s}_{tb}")
            for tt in range(TB):
                t = tb * TB + tt
                r0 = t * TROWS
                cvp = cv_ps.tile([128, TROWS * W], F32, tag="cvp",
                                 name=f"cvp{s}_{t}")
                i = 0
                for dx in range(3):
                    for rho in range(2):
                        m0 = 2 * r0 + 2 * rho
                        last_mm[0] = nc.tensor.matmul(
                            out=cvp,
                            lhsT=stat[:, dx * 2 + rho,
                                      s * 128 : (s + 1) * 128],
                            rhs=xd[s][:, m0 : m0 + 2 * TROWS - 1 : 2,
                                      dx : dx + W],
                            start=(i == 0), stop=(i == 5),
                        )
                        i += 1
                nc.scalar.copy(
                    out=ost[:, tt * TROWS * W : (tt + 1) * TROWS * W],
                    in_=cvp,
                )
            o5 = out_ap[s].rearrange("hp co r w -> (hp co) r w")
            nc.sync.dma_start(
                out=o5[:, tb * TB * TROWS : (tb + 1) * TB * TROWS, :],
                in_=ost.rearrange("p (r w) -> p r w", r=TB * TROWS, w=W),
            )
            if after_pair is not None:
                after_pair(tb)

    # ================= schedule =================
    nc.vector.memset(h1T8, 0.0)
    for s in (0, 1):
        load_x_j0(s)
    for t in range(NTAP):
        load_w2(t)
    for s in (2, 3):
        load_x_j0(s)
    load_x_j1_dma(3)

    for s in (0, 1, 2, 3):
        edge_memset(s)

    # DVE: s0 trees, dup-s0, s1 trees -> h1 A; Scalar: c0c1 accum chunks;
    # Pool: dup s1 then s2; s3 j1 via DMA
    feat_chunk(0, 2, "vector")
    feat_chunk(0, 3, "vector")
    dup_j1(0, "vector")
    feat_chunk(0, 0, "scalar")
    feat_chunk(0, 1, "scalar")
    feat_chunk(1, 0, "scalar")
    feat_chunk(1, 1, "scalar")
    feat_chunk(1, 2, "vector")
    feat_chunk(1, 3, "vector")
    feat_combine(0)
    feat_combine(1)
    h1_compute(0, 2)
    dup_j1(1, "gpsimd")
    dup_j1(2, "gpsimd")

    # wgen pass A (s0, s1) - trails the w2 chunk DMAs; high priority so the
    # scheduler doesn't slot conv-s0 matmuls ahead of the late taps (which
    # would delay assembly -> feat trees -> h1 B)
    with tc.high_priority():
        for t in range(NTAP):
            wgen_tap(t, 0, 2, "A")

    conv_tiles(0, 0, NT)

    # feat s2/s3 trees after conv-s0 emission (cannot overtake assembly-A);
    # level-interleaved so the independent chains pipeline on DVE
    for s in (2, 3):
        feat_trees_interleaved(s)
        feat_combine(s)

    anchor = [None]

    def snap_anchor(tb):
        if tb == 3:
            anchor[0] = last_mm[0]

    conv_tiles(1, 0, NT, after_pair=snap_anchor)

    # h1 B + wgen pass B anchored after conv-s1 tile-pair 3: by then the
    # feat trees are done, so the strictly in-order PE stream never parks.
    tc.cur_priority += 100000
    s1_last = anchor[0]
    h1_compute(2, 4, relu_eng="vector", dep=s1_last)
    for t in range(NTAP):
        wgen_tap(t, 2, 4, "B", dep=s1_last)
    for s in (2, 3):
        conv_tiles(s, 0, NT)


_CACHE = {}


def build_nc():
    if "nc" in _CACHE:
        return _CACHE["nc"], _CACHE["aps"]
    nc = bacc.Bacc("TRN2", debug=False, num_devices=NCORES)
    aps = {
        "x": nc.dram_tensor("x", [BSH, CIN, H, WP], BF16, kind="ExternalInput").ap(),
        "w1": nc.dram_tensor("w1", [CIN, HID], F32, kind="ExternalInput").ap(),
        "b1": nc.dram_tensor("b1", [HID, 1], F32, kind="ExternalInput").ap(),
        "w2": nc.dram_tensor("w2", [HID, JTOT], FP8, kind="ExternalInput").ap(),
        "b2s": nc.dram_tensor("b2s", [128, 6, 512], BF16, kind="ExternalInput").ap(),
        "out": nc.dram_tensor(
            "out", [BSH, 2, COUT, R2, W], BF16, kind="ExternalOutput"
        ).ap(),
    }
    with tile.TileContext(nc) as tc, ExitStack() as ctx:
        build_kernel_body(nc, tc, ctx, aps)
    nc.compile()
    _CACHE["nc"] = nc
    _CACHE["aps"] = aps
    return nc, aps


def make_in_maps(x, w1, b1, w2, b2):
    import ml_dtypes

    E4 = ml_dtypes.float8_e4m3fn
    x = np.asarray(x, dtype=np.float32)
    xpad = np.zeros((B, CIN, H, WP), dtype=ml_dtypes.bfloat16)
    xpad[:, :, :, 1 : W + 1] = x.astype(ml_dtypes.bfloat16)

    w1s = np.ascontiguousarray(np.asarray(w1, dtype=np.float32) / HW)
    b1r = np.ascontiguousarray(np.asarray(b1, dtype=np.float32)).reshape(HID, 1)

    # w2 tap-major reorder: col' = ((tap*32 + cohi)*2 + colow)*64 + ci
    # maps orig col (co=cohi*2+colow, ci, o=tap)
    w2v = np.asarray(w2, dtype=np.float32).reshape(HID, COUT, CIN, NTAP)
    w2r = np.transpose(w2v, (0, 3, 1, 2))  # [hid, tap, co, ci]
    # co -> (cohi, colow): interleave so chunk cols = (colow, ci)
    w2r = w2r.reshape(HID, NTAP, COUT // 2, 2, CIN)  # [hid, tap, cohi, colow, ci]
    w2r = np.ascontiguousarray(w2r.reshape(HID, JTOT).astype(E4))

    # b2 stationaries [128=(j,ci), 6=(dx,rho), 512=(s,hp,co)] bf16
    b2v = np.asarray(b2, dtype=np.float32).reshape(COUT, CIN, K, K)
    b2st = np.zeros((128, 6, BSH, 2, COUT), dtype=np.float32)
    for dy in range(3):
        for rho, j, hp in DY_SLOTS[dy]:
            for dx in range(3):
                # [ci, co] block
                blk = b2v[:, :, dy, dx].T
                b2st[j * 64 : (j + 1) * 64, dx * 2 + rho, :, hp, :] = blk[
                    :, None, :
                ]
    b2st = np.ascontiguousarray(
        b2st.reshape(128, 6, 512).astype(ml_dtypes.bfloat16)
    )

    in_maps = []
    for c in range(NCORES):
        in_maps.append(
            {
                "x": np.ascontiguousarray(xpad[c * BSH : (c + 1) * BSH]),
                "w1": w1s,
                "b1": b1r,
                "w2": w2r,
                "b2s": b2st,
            }
        )
    return in_maps


def kernel(x, w1, b1, w2, b2, _trace=False, _results_out=None):
    nc, _ = build_nc()
    in_maps = make_in_maps(x, w1, b1, w2, b2)
    res = run_bass_kernel_spmd(
        nc, in_maps, core_ids=list(range(NCORES)), trace=_trace
    )
    if _results_out is not None:
        _results_out.append(res)
    # out per core: [BSH, 2, COUT, R2, W] bf16 -> [BSH, COUT, H, W] f32
    parts = []
    for r in res.results:
        o = np.asarray(r["out"]).astype(np.float32)
        o = o.transpose(0, 2, 3, 1, 4).reshape(BSH, COUT, H, W)
        parts.append(o)
    return np.concatenate(parts, axis=0)


if __name__ == "__main__":
    rng = np.random.default_rng(0)
    ins = {
        "x": rng.standard_normal((B, CIN, H, W)).astype(np.float32),
        "w1": (rng.standard_normal((CIN, HID)) * 0.05).astype(np.float32),
        "b1": (rng.standard_normal((HID,)) * 0.05).astype(np.float32),
        "w2": (rng.standard_normal((HID, JTOT)) * 0.05).astype(np.float32),
        "b2": (rng.standard_normal((JTOT,)) * 0.05).astype(np.float32),
    }
    out = kernel(**ins)
    print("out", out.shape, out.dtype, np.abs(out).mean())
